# revision 16
# baseline (speedup 1.0000x reference)
"""ConditioningEncoder Trainium2 kernel.

Data-parallel over batch (B=8 -> 8 NeuronCores). Each core gets the batch
ROTATED so its own element is index 0; phoneme/f0 branches are computed
full-batch on every core (BatchNorm stats local), the pc/cb1/cb2 BatchNorms
exchange sum/sumsq via 3 tiny AllReduces. Final bilinear freq upsample is a
per-plane fused multiply-add; 40 of the 80 freq planes are pure copies.
"""
import sys

if "/opt/trn_rl_repo" not in sys.path:
    sys.path.insert(0, "/opt/trn_rl_repo")

import numpy as np

import concourse.bass as bass
import concourse.bacc as bacc
import concourse.tile as tile
import concourse.mybir as mybir
from concourse import bass_utils

B, P, T, H, E, NUM_PH, FREQ = 8, 64, 1024, 256, 256, 100, 80
N_CORES = 8
F32 = mybir.dt.float32
EPS = 1e-5
ALU = mybir.AluOpType
ACTF = mybir.ActivationFunctionType


def _interp_w1():
    # jax.image.resize 'bilinear', half-pixel centers, 2 -> 80 rows:
    # out[f] = (1-w1[f])*row0 + w1[f]*row1 with w1 = clip((f+0.5)/40 - 0.5, 0, 1)
    s = (np.arange(FREQ, dtype=np.float64) + 0.5) * (2.0 / FREQ) - 0.5
    return np.clip(s, 0.0, 1.0)


W1 = _interp_w1()


def _build(nc, debug=False):
    """Emit the SPMD program. Returns dict of input names -> shapes."""
    dram = {}

    def din(name, shape):
        dram[name] = nc.dram_tensor(name, list(shape), F32, kind="ExternalInput").ap()
        return dram[name]

    # --- per-core data (differs between cores)
    f0r = din("f0r", (B, T))          # rotated: row 0 = own batch element
    idsr = din("idsr", (1, B * P))    # rotated phoneme ids, flattened, as f32
    midir = din("midir", (1, T))      # own midi row, f32
    cum2 = din("cum2", (P, 2))        # col0 = cumsum(dur), col1 = shifted cumsum (own b)
    # --- replicated weights / constants
    phemb = din("phemb", (NUM_PH, E))
    w1ph = din("w1ph", (2, 128, 3, 2, 128))
    w2ph = din("w2ph", (2, 128, 3, 2, 128))
    f0w1 = din("f0w1", (3, 128))
    f0w2 = din("f0w2", (128, 3, 2, 128))
    ptemb = din("ptemb", (128, H))
    pcw = din("pcw", (4, 128, 3, 2, 128))
    cbw1 = din("cbw1", (4, 128, 3, 2, 128))
    cbw2 = din("cbw2", (2, 128, 3, 2, 128))
    fpw = din("fpw", (2, 128, 4, 128))
    fpb = din("fpb", (128, 4))
    bnaff = din("bnaff", (128, 26))
    ident = din("ident", (128, 128))
    ones1 = din("ones1", (1, 128))
    iotac = din("iotac", (128, 1))
    trow = din("trow", (1, T))

    out = nc.dram_tensor("out", [H, FREQ, T], F32, kind="ExternalOutput").ap()
    dbg = {}
    if debug:
        for nm, shp in [("d_oph", (NUM_PH, B * P)), ("d_opt", (128, T)),
                        ("d_x2", (2, 128, B * P)), ("d_smat", (P, T)),
                        ("d_g1p", (128, B, T)), ("d_pcin", (4, 128, T)),
                        ("d_cb1in", (4, 128, T)), ("d_cb2in", (2, 128, T)),
                        ("d_fpin", (2, 128, T)), ("d_ud", (4, 128, T)),
                        ("d_st", (3, 128, 4))]:
            dbg[nm] = nc.dram_tensor(nm, list(shp), F32, kind="ExternalOutput").ap()

    with tile.TileContext(nc) as tc:
        with (
            tc.tile_pool(name="sb", bufs=1) as sb,
            tc.tile_pool(name="stage", bufs=3) as stg,
            tc.tile_pool(name="ps", bufs=4, space="PSUM") as pss,
            tc.tile_pool(name="pb", bufs=2, space="PSUM") as psb,
            tc.tile_pool(name="dr", bufs=1, space="DRAM") as drp,
        ):
            tinybank = sb.tile([128, 256], F32, tag="tinybank", name="tinybank")
            tiny_n = [0]

            def tiny():
                i = tiny_n[0]
                tiny_n[0] += 1
                assert i < 256
                return tinybank[:, i : i + 1]

            def load(name, ap, shape, tag):
                t = sb.tile(list(shape), F32, tag=tag)
                nc.sync.dma_start(t[:], ap)
                return t

            # ---------- load constants & weights ----------
            ids_sb = load("idsr", idsr[:], (1, B * P), "idsr")
            midi_sb = load("midir", midir[:], (1, T), "midir")
            cum_sb = load("cum2", cum2[:], (P, 2), "cum2")
            phemb_sb = load("phemb", phemb[:], (NUM_PH, E), "phemb")
            w1ph_sb = [load("w1ph", w1ph[i], (128, 3, 2, 128), f"w1ph{i}") for i in range(2)]
            w2ph_sb = [load("w2ph", w2ph[i], (128, 3, 2, 128), f"w2ph{i}") for i in range(2)]
            f0w1_sb = load("f0w1", f0w1[:], (3, 128), "f0w1")
            f0w2_sb = load("f0w2", f0w2[:], (128, 3, 2, 128), "f0w2")
            ptemb_sb = load("ptemb", ptemb[:], (128, H), "ptemb")
            pcw_sb = [load("pcw", pcw[i], (128, 3, 2, 128), f"pcw{i}") for i in range(4)]
            cbw1_sb = [load("cbw1", cbw1[i], (128, 3, 2, 128), f"pcw{i}") for i in range(4)]
            cbw2_sb = [load("cbw2", cbw2[i], (128, 3, 2, 128), f"w1ph{i}") for i in range(2)]
            fpw_sb = [load("fpw", fpw[i], (128, 4, 128), f"w2ph{i}") for i in range(2)]
            fpb_sb = load("fpb", fpb[:], (128, 4), "fpb")
            aff_sb = load("bnaff", bnaff[:], (128, 26), "bnaff")
            id_sb = load("ident", ident[:], (128, 128), "ident")
            on_sb = load("ones1", ones1[:], (1, 128), "ones1")
            io_sb = load("iotac", iotac[:], (128, 1), "iotac")
            tr_sb = load("trow", trow[:], (1, T), "trow")

            eps_sb = sb.tile([128, 1], F32, tag="eps", name="eps")
            nc.vector.memset(eps_sb[:], EPS)

            # bnaff column index helper
            aff_idx = {}
            _i = 0
            for lname, nct in [("ph1", 2), ("ph2", 2), ("f01", 1), ("f02", 2),
                               ("pc", 2), ("cb1", 2), ("cb2", 2)]:
                aff_idx[lname] = _i  # g columns at _i.._i+nct-1, beta at _i+nct.._i+2nct-1
                _i += 2 * nct

            def aff_cols(lname, nct, ct):
                base = aff_idx[lname]
                g = aff_sb[:, base + ct : base + ct + 1]
                be = aff_sb[:, base + nct + ct : base + nct + ct + 1]
                return g, be

            def bn_coeffs(gsum, gsq, n, g_col, b_col):
                """From per-channel sum/sumsq columns -> (scale, bias) columns."""
                mean = tiny()
                nc.vector.tensor_scalar_mul(mean[:], gsum, 1.0 / n)
                ex2 = tiny()
                nc.vector.tensor_scalar_mul(ex2[:], gsq, 1.0 / n)
                m2 = tiny()
                nc.scalar.activation(m2[:], mean[:], ACTF.Square)
                var = tiny()
                nc.vector.tensor_sub(var[:], ex2[:], m2[:])
                std = tiny()
                nc.scalar.activation(std[:], var[:], ACTF.Sqrt, bias=eps_sb[:])
                inv = tiny()
                nc.vector.reciprocal(inv[:], std[:])
                scale = tiny()
                nc.vector.tensor_mul(scale[:], inv[:], g_col)
                mb = tiny()
                nc.vector.tensor_mul(mb[:], mean[:], scale[:])
                bias = tiny()
                nc.vector.tensor_sub(bias[:], b_col, mb[:])
                return scale, bias

            def leaky_inplace(ap_):
                # x <- max(0.1*x, x)
                nc.vector.scalar_tensor_tensor(ap_, ap_, 0.1, ap_, ALU.mult, ALU.max)

            # =========================================================
            # Phase A-f0 (full batch, replicated; local BN stats)
            # =========================================================
            # f0 shifted rows: partition k holds f0n[bb, t+k-1] along free dim (b, t)
            f0sh = sb.tile([3, B, T], F32, tag="f0sh", name="f0sh")
            nc.vector.memset(f0sh[:], 0.0)
            f0r3 = f0r[:].rearrange("(o b) t -> o b t", o=1)
            nc.sync.dma_start(f0sh[0:1, :, 1:T], f0r3[:, :, 0 : T - 1])
            nc.sync.dma_start(f0sh[1:2, :, :], f0r3[:, :, :])
            nc.sync.dma_start(f0sh[2:3, :, 0 : T - 1], f0r3[:, :, 1:T])
            # f0n = relu(f0/500) in place
            nc.scalar.activation(f0sh[:], f0sh[:], ACTF.Relu, scale=1.0 / 500.0)

            # conv1: 128 out channels, K = 3 (taps as contraction)
            g1p = sb.tile([128, B, T + 2], F32, tag="g1p", name="g1p")
            nc.vector.memset(g1p[:, :, 0:1], 0.0)
            nc.vector.memset(g1p[:, :, T + 1 : T + 2], 0.0)
            c1sum = sb.tile([128, 16], F32, tag="c1sum", name="c1sum")
            c1sq = sb.tile([128, 16], F32, tag="c1sq", name="c1sq")
            for bb in range(B):
                for hh in range(2):
                    i = bb * 2 + hh
                    ps = pss.tile([128, 512], F32, tag="ps", name="ps")
                    nc.tensor.matmul(ps[:], f0w1_sb[:],
                                     f0sh[0:3, bb, hh * 512 : hh * 512 + 512])
                    # copy -> g1p interior with accumulated per-channel sum
                    nc.vector.tensor_scalar(
                        g1p[:, bb, 1 + hh * 512 : 513 + hh * 512], ps[:],
                        0.0, None, ALU.add, ALU.add, accum_out=c1sum[:, i : i + 1],
                    )
                    nc.scalar.activation(ps[:], ps[:], ACTF.Square, accum_out=c1sq[:, i : i + 1])
            s1 = tiny()
            nc.vector.reduce_sum(s1[:], c1sum[:], axis=mybir.AxisListType.X)
            q1 = tiny()
            nc.vector.reduce_sum(q1[:], c1sq[:], axis=mybir.AxisListType.X)
            g_c, b_c = aff_cols("f01", 1, 0)
            sc, bi = bn_coeffs(s1[:], q1[:], B * T, g_c, b_c)
            gint = g1p[:, :, 1 : T + 1]
            nc.scalar.activation(gint, gint, ACTF.Identity, bias=bi[:], scale=sc[:])
            leaky_inplace(gint)

            # conv2: 256 out, cin 128, 3 taps
            g2raw = [sb.tile([128, T], F32, tag=f"g2raw{ct}", name=f"g2raw{ct}") for ct in range(2)]
            c2sum = [sb.tile([128, 16], F32, tag=f"c2sum{ct}", name=f"c2sum{ct}") for ct in range(2)]
            c2sq = [sb.tile([128, 16], F32, tag=f"c2sq{ct}", name=f"c2sq{ct}") for ct in range(2)]
            for ct in range(2):
                for bb in range(B):
                    for hh in range(2):
                        i = bb * 2 + hh
                        ps = pss.tile([128, 512], F32, tag="ps", name="ps")
                        for k in range(3):
                            nc.tensor.matmul(
                                ps[:], f0w2_sb[:, k, ct, :],
                                g1p[:, bb, hh * 512 + k : hh * 512 + k + 512],
                                start=(k == 0), stop=(k == 2),
                            )
                        nc.vector.tensor_reduce(
                            c2sum[ct][:, i : i + 1], ps[:],
                            axis=mybir.AxisListType.X, op=ALU.add,
                        )
                        if bb == 0:
                            nc.vector.tensor_copy(
                                g2raw[ct][:, hh * 512 : hh * 512 + 512], ps[:])
                        nc.scalar.activation(ps[:], ps[:], ACTF.Square,
                                             accum_out=c2sq[ct][:, i : i + 1])
            # pc conv input tiles: [pf0, pf1, g0, g1], padded length T+2
            pcin = [sb.tile([128, T + 2], F32, tag=f"pcin{i}", name=f"pcin{i}") for i in range(4)]
            for i in range(4):
                nc.vector.memset(pcin[i][:, 0:1], 0.0)
                nc.vector.memset(pcin[i][:, T + 1 : T + 2], 0.0)
            for ct in range(2):
                s2 = tiny()
                nc.vector.reduce_sum(s2[:], c2sum[ct][:], axis=mybir.AxisListType.X)
                q2 = tiny()
                nc.vector.reduce_sum(q2[:], c2sq[ct][:], axis=mybir.AxisListType.X)
                g_c, b_c = aff_cols("f02", 2, ct)
                sc, bi = bn_coeffs(s2[:], q2[:], B * T, g_c, b_c)
                dst = pcin[2 + ct][:, 1 : T + 1]
                nc.scalar.activation(dst, g2raw[ct][:], ACTF.Identity, bias=bi[:], scale=sc[:])
                leaky_inplace(dst)

            # =========================================================
            # Phase A-ph (full batch, replicated; local BN stats)
            # =========================================================
            psI = pss.tile([128, 512], F32, tag="ps", name="ps")
            nc.tensor.matmul(psI[0:NUM_PH, :], on_sb[:, 0:NUM_PH], ids_sb[:])
            oph = sb.tile([NUM_PH, B * P], F32, tag="oph", name="oph")
            nc.vector.tensor_scalar(oph[:], psI[0:NUM_PH, :], io_sb[0:NUM_PH, :],
                                    None, ALU.is_equal)
            x0p = [sb.tile([128, B, P + 2], F32, tag=f"x0p{i}", name=f"x0p{i}") for i in range(2)]
            x1p = [sb.tile([128, B, P + 2], F32, tag=f"x1p{i}", name=f"x1p{i}") for i in range(2)]
            for i in range(2):
                nc.vector.memset(x0p[i][:, :, 0:1], 0.0)
                nc.vector.memset(x0p[i][:, :, P + 1 : P + 2], 0.0)
                nc.vector.memset(x1p[i][:, :, 0:1], 0.0)
                nc.vector.memset(x1p[i][:, :, P + 1 : P + 2], 0.0)
            for et in range(2):
                ps = pss.tile([128, 512], F32, tag="ps", name="ps")
                nc.tensor.matmul(ps[:], phemb_sb[:, et * 128 : et * 128 + 128], oph[:])
                nc.vector.tensor_copy(
                    x0p[et][:, :, 1 : P + 1],
                    ps[:].rearrange("p (b l) -> p b l", b=B))
            phsum = sb.tile([128, 4], F32, tag="phsum", name="phsum")  # cols: conv1 ct0/ct1, conv2 ct0/ct1
            phsq = sb.tile([128, 4], F32, tag="phsq", name="phsq")
            for ct in range(2):
                ps = pss.tile([128, 512], F32, tag="ps", name="ps")
                psv = ps[:].rearrange("p (b l) -> p b l", b=B)
                n_mm = 0
                for et in range(2):
                    for k in range(3):
                        nc.tensor.matmul(
                            psv, w1ph_sb[et][:, k, ct, :],
                            x0p[et][:, :, k : k + P],
                            start=(n_mm == 0), stop=(n_mm == 5),
                        )
                        n_mm += 1
                nc.vector.tensor_scalar(
                    x1p[ct][:, :, 1 : P + 1], psv, 0.0, None, ALU.add, ALU.add,
                    accum_out=phsum[:, ct : ct + 1])
                nc.scalar.activation(ps[:], ps[:], ACTF.Square,
                                     accum_out=phsq[:, ct : ct + 1])
            for ct in range(2):
                g_c, b_c = aff_cols("ph1", 2, ct)
                sc, bi = bn_coeffs(phsum[:, ct : ct + 1], phsq[:, ct : ct + 1],
                                   B * P, g_c, b_c)
                dst = x1p[ct][:, :, 1 : P + 1]
                nc.scalar.activation(dst, dst, ACTF.Relu, bias=bi[:], scale=sc[:])
            x2sb = [sb.tile([128, B, P], F32, tag=f"x2sb{i}", name=f"x2sb{i}") for i in range(2)]
            for ct in range(2):
                ps = pss.tile([128, 512], F32, tag="ps", name="ps")
                psv = ps[:].rearrange("p (b l) -> p b l", b=B)
                n_mm = 0
                for et in range(2):
                    for k in range(3):
                        nc.tensor.matmul(
                            psv, w2ph_sb[et][:, k, ct, :],
                            x1p[et][:, :, k : k + P],
                            start=(n_mm == 0), stop=(n_mm == 5),
                        )
                        n_mm += 1
                nc.vector.tensor_scalar(
                    x2sb[ct][:], psv, 0.0, None, ALU.add, ALU.add,
                    accum_out=phsum[:, 2 + ct : 3 + ct])
                nc.scalar.activation(ps[:], ps[:], ACTF.Square,
                                     accum_out=phsq[:, 2 + ct : 3 + ct])
            for ct in range(2):
                g_c, b_c = aff_cols("ph2", 2, ct)
                sc, bi = bn_coeffs(phsum[:, 2 + ct : 3 + ct], phsq[:, 2 + ct : 3 + ct],
                                   B * P, g_c, b_c)
                nc.scalar.activation(x2sb[ct][:], x2sb[ct][:], ACTF.Relu,
                                     bias=bi[:], scale=sc[:])
            # transpose own-b (index 0) slice -> x2T [64, 256]
            x2T = sb.tile([P, H], F32, tag="x2T", name="x2T")
            for ct in range(2):
                ps = pss.tile([128, 512], F32, tag="ps", name="ps")
                nc.tensor.transpose(ps[0:P, 0:128], x2sb[ct][:, 0, :], id_sb[:])
                nc.vector.tensor_copy(x2T[:, ct * 128 : ct * 128 + 128], ps[0:P, 0:128])
            # length-regulator select matrix S [64, 1024]
            tbf = sb.tile([P, T], F32, tag="tbf", name="tbf")
            for hh in range(2):
                ps = pss.tile([128, 512], F32, tag="ps", name="ps")
                nc.tensor.matmul(ps[0:P, :], on_sb[:, 0:P], tr_sb[:, hh * 512 : hh * 512 + 512])
                nc.vector.tensor_copy(tbf[:, hh * 512 : hh * 512 + 512], ps[0:P, :])
            smat = sb.tile([P, T], F32, tag="smat", name="smat")
            nc.vector.tensor_scalar(smat[:], tbf[:], cum_sb[:, 1:2], None, ALU.is_ge)
            nc.vector.scalar_tensor_tensor(smat[:], tbf[:], cum_sb[:, 0:1], smat[:],
                                           ALU.is_lt, ALU.mult)
            # x_exp -> cb1 input tiles 0..1 (padded)
            cb1in = [sb.tile([128, T + 2], F32, tag=f"cb1in{i}", name=f"cb1in{i}") for i in range(4)]
            for i in range(4):
                nc.vector.memset(cb1in[i][:, 0:1], 0.0)
                nc.vector.memset(cb1in[i][:, T + 1 : T + 2], 0.0)
            for ht in range(2):
                for tc_ in range(2):
                    ps = pss.tile([128, 512], F32, tag="ps", name="ps")
                    nc.tensor.matmul(ps[:], x2T[:, ht * 128 : ht * 128 + 128],
                                     smat[:, tc_ * 512 : tc_ * 512 + 512])
                    nc.vector.tensor_copy(
                        cb1in[ht][:, 1 + tc_ * 512 : 513 + tc_ * 512], ps[:])

            # =========================================================
            # Phase B-pt: pitch embedding for own midi row
            # =========================================================
            opt_ = sb.tile([128, T], F32, tag="opt", name="opt")
            for hh in range(2):
                ps = pss.tile([128, 512], F32, tag="ps", name="ps")
                nc.tensor.matmul(ps[:], on_sb[:], midi_sb[:, hh * 512 : hh * 512 + 512])
                nc.vector.tensor_scalar(opt_[:, hh * 512 : hh * 512 + 512], ps[:],
                                        io_sb[:], None, ALU.is_equal)
            for ht in range(2):
                for tc_ in range(2):
                    ps = pss.tile([128, 512], F32, tag="ps", name="ps")
                    nc.tensor.matmul(ps[:], ptemb_sb[:, ht * 128 : ht * 128 + 128],
                                     opt_[:, tc_ * 512 : tc_ * 512 + 512])
                    nc.vector.tensor_copy(
                        pcin[ht][:, 1 + tc_ * 512 : 513 + tc_ * 512], ps[:])

            # =========================================================
            # Sharded convs with AllReduce'd BN stats
            # =========================================================
            st_dumps = {}

            def conv_stats_ar(tag, w_sb, in_tiles, n_ci, lname, out_pad_tiles,
                              interior):
                """3-tap conv over padded input tiles -> psb tiles; sum/sumsq ->
                AllReduce -> BN+leaky apply into out_pad_tiles[ct][interior]."""
                pbt = []
                st = sb.tile([128, 4], F32, tag=f"st_{tag}", name=f"st_{tag}")
                for ct in range(2):
                    pb = psb.tile([128, T], F32, tag="pb", name="pb")
                    pbt.append(pb)
                    for tc_ in range(2):
                        n_mm = 0
                        for ci in range(n_ci):
                            for k in range(3):
                                nc.tensor.matmul(
                                    pb[:, tc_ * 512 : tc_ * 512 + 512],
                                    w_sb[ci][:, k, ct, :],
                                    in_tiles[ci][:, tc_ * 512 + k : tc_ * 512 + k + 512],
                                    start=(n_mm == 0), stop=(n_mm == 3 * n_ci - 1),
                                )
                                n_mm += 1
                    nc.vector.tensor_reduce(st[:, 2 * ct : 2 * ct + 1], pb[:],
                                            axis=mybir.AxisListType.X, op=ALU.add)
                    scr = stg.tile([128, T], F32, tag="scr1024", name="scr1024", bufs=2)
                    nc.scalar.activation(scr[:], pb[:], ACTF.Square,
                                         accum_out=st[:, 2 * ct + 1 : 2 * ct + 2])
                arin = drp.tile([128, 4], F32, tag=f"arin_{tag}", name=f"arin_{tag}")
                arout = drp.tile([128, 4], F32, tag=f"arout_{tag}", name=f"arout_{tag}")
                nc.gpsimd.dma_start(arin[:], st[:])
                nc.gpsimd.collective_compute(
                    "AllReduce", ALU.add,
                    replica_groups=[list(range(N_CORES))],
                    ins=[arin.opt()], outs=[arout.opt()],
                )
                gst = sb.tile([128, 4], F32, tag=f"gst_{tag}", name=f"gst_{tag}")
                nc.gpsimd.dma_start(gst[:], arout[:])
                st_dumps[tag] = gst
                for ct in range(2):
                    g_c, b_c = aff_cols(lname, 2, ct)
                    sc, bi = bn_coeffs(gst[:, 2 * ct : 2 * ct + 1],
                                       gst[:, 2 * ct + 1 : 2 * ct + 2],
                                       N_CORES * T, g_c, b_c)
                    dst = out_pad_tiles[ct][:, interior[0] : interior[1]]
                    nc.scalar.activation(dst, pbt[ct][:], ACTF.Identity,
                                         bias=bi[:], scale=sc[:])
                    leaky_inplace(dst)

            cb2in = [sb.tile([128, T + 2], F32, tag=f"x0p{i}", name=f"cb2in{i}") for i in range(2)]
            for i in range(2):
                nc.vector.memset(cb2in[i][:, 0:1], 0.0)
                nc.vector.memset(cb2in[i][:, T + 1 : T + 2], 0.0)
            fpin = [sb.tile([128, T], F32, tag=f"x1p{i}", name=f"fpin{i}") for i in range(2)]

            conv_stats_ar("pc", pcw_sb, pcin, 4, "pc", cb1in[2:], (1, T + 1))
            conv_stats_ar("cb1", cbw1_sb, cb1in, 4, "cb1", cb2in, (1, T + 1))
            conv_stats_ar("cb2", cbw2_sb, cb2in, 2, "cb2", fpin, (0, T))

            # =========================================================
            # fp 1x1 conv -> u (ch 0..255) and d = v - u (ch 256..511)
            # =========================================================
            ud_tags = ["g2raw0", "g2raw1", "x2sb0", "x2sb1"]
            ud = [sb.tile([128, T], F32, tag=ud_tags[i], name=f"ud{i}") for i in range(4)]
            for g in range(2):
                for ct2 in range(2):
                    ct = g * 2 + ct2
                    pb = psb.tile([128, T], F32, tag="pb", name="pb")
                    for tc_ in range(2):
                        for ci in range(2):
                            nc.tensor.matmul(
                                pb[:, tc_ * 512 : tc_ * 512 + 512],
                                fpw_sb[ci][:, ct, :],
                                fpin[ci][:, tc_ * 512 : tc_ * 512 + 512],
                                start=(ci == 0), stop=(ci == 1),
                            )
                    nc.scalar.activation(ud[ct][:], pb[:], ACTF.Identity,
                                         bias=fpb_sb[:, ct : ct + 1])
            vv_tags = ["opt", "oph"]
            vv = [sb.tile([128, T], F32, tag=vv_tags[i], name=f"vv{i}") for i in range(2)]
            for ht in range(2):
                nc.vector.tensor_add(vv[ht][:], ud[ht][:], ud[2 + ht][:])

            if debug:
                nc.sync.dma_start(dbg["d_oph"][:], oph[:])
                nc.sync.dma_start(dbg["d_opt"][:], opt_[:])
                for ct in range(2):
                    nc.sync.dma_start(
                        dbg["d_x2"][ct], x2sb[ct][:].rearrange("p b l -> p (b l)"))
                nc.sync.dma_start(dbg["d_smat"][:], smat[:])
                nc.sync.dma_start(dbg["d_g1p"][:], g1p[:, :, 1 : T + 1])
                for i in range(4):
                    nc.sync.dma_start(dbg["d_pcin"][i], pcin[i][:, 1 : T + 1])
                    nc.sync.dma_start(dbg["d_cb1in"][i], cb1in[i][:, 1 : T + 1])
                    nc.sync.dma_start(dbg["d_ud"][i], ud[i][:])
                for i in range(2):
                    nc.sync.dma_start(dbg["d_cb2in"][i], cb2in[i][:, 1 : T + 1])
                    nc.sync.dma_start(dbg["d_fpin"][i], fpin[i][:])
                nc.sync.dma_start(dbg["d_st"][0], st_dumps["pc"][:])
                nc.sync.dma_start(dbg["d_st"][1], st_dumps["cb1"][:])
                nc.sync.dma_start(dbg["d_st"][2], st_dumps["cb2"][:])

            # =========================================================
            # interp + write out: out[h, f, t] = u + W1[f] * d
            # =========================================================
            for ht in range(2):
                u, d, v = ud[ht], ud[2 + ht], vv[ht]
                for f in range(FREQ):
                    w1f = float(W1[f])
                    if w1f == 0.0:
                        src = u
                    elif w1f == 1.0:
                        src = v
                    else:
                        s = stg.tile([128, T], F32, tag="stage", name="stage")
                        nc.vector.scalar_tensor_tensor(s[:], d[:], w1f, u[:],
                                                       ALU.mult, ALU.add)
                        src = s
                    nc.sync.dma_start(out[ht * 128 : ht * 128 + 128, f, :], src[:])

    return dram


_COMPILED = {}


def _get_compiled(debug=False):
    if debug not in _COMPILED:
        nc = bacc.Bacc("TRN2", target_bir_lowering=False, debug=False,
                       num_devices=N_CORES)
        _build(nc, debug=debug)
        nc.compile()
        _COMPILED[debug] = nc
    return _COMPILED[debug]


def _host_prep(phoneme_ids, phoneme_durations, midi_pitch, f0, params):
    p = {k: np.asarray(v, dtype=np.float32) for k, v in params.items()}
    ids = np.asarray(phoneme_ids).astype(np.int64)
    dur = np.asarray(phoneme_durations).astype(np.int64)
    midi = np.asarray(midi_pitch).astype(np.int64)
    f0 = np.asarray(f0, dtype=np.float32)

    def conv_lhsT(w, n_ci, n_ct):
        # [C_out, C_in, K] -> [ci, 128, K, ct, 128]
        wt = np.ascontiguousarray(np.transpose(w, (1, 2, 0)))  # [cin, k, cout]
        return wt.reshape(n_ci, 128, 3, n_ct, 128)

    shared = {
        "phemb": p["ph_emb"],
        "w1ph": conv_lhsT(p["ph_w1"], 2, 2),
        "w2ph": conv_lhsT(p["ph_w2"], 2, 2),
        "f0w1": np.ascontiguousarray(p["f0_w1"][:, 0, :].T),
        "f0w2": conv_lhsT(p["f0_w2"], 1, 2).reshape(128, 3, 2, 128),
        "ptemb": p["pt_emb"],
        "pcw": conv_lhsT(p["pc_w"], 4, 2),
        "cbw1": conv_lhsT(p["cb_w1"], 4, 2),
        "cbw2": conv_lhsT(p["cb_w2"], 2, 2),
        "ident": np.eye(128, dtype=np.float32),
        "ones1": np.ones((1, 128), dtype=np.float32),
        "iotac": np.arange(128, dtype=np.float32).reshape(128, 1),
        "trow": np.arange(T, dtype=np.float32).reshape(1, T),
    }
    fpT = p["fp_w"][:, :, 0].T  # [cin 256, cout 512]
    fpmod = np.concatenate([fpT[:, :H], fpT[:, H:] - fpT[:, :H]], axis=1)
    shared["fpw"] = np.ascontiguousarray(fpmod).reshape(2, 128, 4, 128)
    fb = np.concatenate([p["fp_b"][:H], p["fp_b"][H:] - p["fp_b"][:H]])
    shared["fpb"] = np.ascontiguousarray(fb.reshape(4, 128).T)

    rows = []
    for gk, bk, nct in [("ph_g1", "ph_be1", 2), ("ph_g2", "ph_be2", 2),
                        ("f0_g1", "f0_be1", 1), ("f0_g2", "f0_be2", 2),
                        ("pc_g", "pc_be", 2), ("cb_g1", "cb_be1", 2),
                        ("cb_g2", "cb_be2", 2)]:
        rows.append(p[gk].reshape(nct, 128))
        rows.append(p[bk].reshape(nct, 128))
    shared["bnaff"] = np.ascontiguousarray(np.concatenate(rows, 0).T)  # [128, 26]

    in_maps = []
    for c in range(N_CORES):
        rot = [(c + i) % B for i in range(B)]
        cum = np.maximum(dur[c], 0).cumsum().astype(np.float32)
        cumprev = np.concatenate([[0.0], cum[:-1]]).astype(np.float32)
        m = dict(shared)
        m["f0r"] = np.ascontiguousarray(f0[rot])
        m["idsr"] = np.ascontiguousarray(ids[rot].reshape(1, B * P)).astype(np.float32)
        m["midir"] = np.clip(midi[c], 0, 127).reshape(1, T).astype(np.float32)
        m["cum2"] = np.stack([cum, cumprev], axis=1)
        in_maps.append({k: np.ascontiguousarray(v, dtype=np.float32) for k, v in m.items()})
    return in_maps


def kernel(phoneme_ids, phoneme_durations, midi_pitch, f0, mel_shape, params,
           _trace=False, _trace_kwargs=None):
    nc = _get_compiled()
    in_maps = _host_prep(phoneme_ids, phoneme_durations, midi_pitch, f0, params)
    res = bass_utils.run_bass_kernel_spmd(
        nc, in_maps, core_ids=list(range(N_CORES)), trace=_trace,
        **(_trace_kwargs or {}),
    )
    out = np.stack([res.results[c]["out"] for c in range(N_CORES)], axis=0)
    if _trace:
        kernel._last_result = res
    return out


# revision 21
# speedup vs baseline: 1.5518x; 1.5518x over previous
"""ConditioningEncoder Trainium2 kernel.

Data-parallel over batch (B=8 -> 8 NeuronCores). Each core gets the batch
ROTATED so its own element is index 0; phoneme conv stats and f0-conv1 stats
are computed full-batch on every core (local BatchNorm stats); f0-conv2 and
the pc/cb1/cb2 convs are sharded per-core with sum/sumsq exchanged via tiny
AllReduces (hidden behind independent compute). Matmuls run as float32r
(full-rate fp32 on the PE). The final bilinear freq upsample is one fused
multiply-add per interior plane; 40 of the 80 freq planes are pure copies
DMA'd straight from the u/v tiles.
"""
import sys

if "/opt/trn_rl_repo" not in sys.path:
    sys.path.insert(0, "/opt/trn_rl_repo")

import numpy as np

import concourse.bass as bass
import concourse.bacc as bacc
import concourse.tile as tile
import concourse.mybir as mybir
from concourse import bass_utils

B, P, T, H, E, NUM_PH, FREQ = 8, 64, 1024, 256, 256, 100, 80
N_CORES = 8
F32 = mybir.dt.float32
F32R = mybir.dt.float32r
EPS = 1e-5
ALU = mybir.AluOpType
ACTF = mybir.ActivationFunctionType


def _interp_w1():
    # jax.image.resize 'bilinear', half-pixel centers, 2 -> 80 rows:
    # out[f] = (1-w1[f])*row0 + w1[f]*row1 with w1 = clip((f+0.5)/40 - 0.5, 0, 1)
    s = (np.arange(FREQ, dtype=np.float64) + 0.5) * (2.0 / FREQ) - 0.5
    return np.clip(s, 0.0, 1.0)


W1 = _interp_w1()


def _build(nc, debug=False):
    dram = {}

    def din(name, shape, dt_=F32):
        dram[name] = nc.dram_tensor(name, list(shape), dt_, kind="ExternalInput").ap()
        return dram[name]

    # --- per-core data (differs between cores)
    f0r = din("f0r", (B, T), F32R)        # rotated: row 0 = own batch element
    idsr = din("idsr", (1, B * P), F32R)  # rotated phoneme ids, flattened, as f32
    midir = din("midir", (1, T), F32R)    # own midi row, f32
    cum2 = din("cum2", (P, 2))            # col0 cumsum(dur), col1 shifted (own b)
    # --- replicated weights / constants
    phemb = din("phemb", (NUM_PH, E), F32R)
    w1ph = din("w1ph", (2, 128, 3, 2, 128), F32R)
    w2ph = din("w2ph", (2, 128, 3, 2, 128), F32R)
    f0w1 = din("f0w1", (3, 128), F32R)
    f0w2 = din("f0w2", (128, 3, 2, 128), F32R)
    ptemb = din("ptemb", (128, H), F32R)
    pcw = din("pcw", (4, 128, 3, 2, 128), F32R)
    cbw1 = din("cbw1", (4, 128, 3, 2, 128), F32R)
    cbw2 = din("cbw2", (2, 128, 3, 2, 128), F32R)
    fpw = din("fpw", (2, 128, 4, 128), F32R)
    fpb = din("fpb", (128, 4))
    bnaff = din("bnaff", (128, 26))
    ident = din("ident", (128, 128))
    ones1 = din("ones1", (1, 128), F32R)
    iotac = din("iotac", (128, 1))
    trow = din("trow", (1, T), F32R)

    out = nc.dram_tensor("out", [H, FREQ, T], F32, kind="ExternalOutput").ap()
    dbg = {}
    if debug:
        for nm, shp in [("d_oph", (NUM_PH, B * P)), ("d_opt", (128, T)),
                        ("d_smat", (P, T)), ("d_g1p", (128, T)),
                        ("d_pcin", (4, 128, T)),
                        ("d_cb1in", (4, 128, T)), ("d_cb2in", (2, 128, T)),
                        ("d_fpin", (2, 128, T)), ("d_ud", (4, 128, T)),
                        ("d_st", (3, 128, 4))]:
            dbg[nm] = nc.dram_tensor(nm, list(shp), F32, kind="ExternalOutput").ap()

    with tile.TileContext(nc) as tc:
        with (
            tc.tile_pool(name="sb", bufs=1) as sb,
            tc.tile_pool(name="stage", bufs=3) as stg,
            tc.tile_pool(name="ps", bufs=4, space="PSUM") as pss,
            tc.tile_pool(name="pb", bufs=2, space="PSUM") as psb,
            tc.tile_pool(name="dr", bufs=1, space="DRAM") as drp,
        ):
            tinybank = sb.tile([128, 256], F32, tag="tinybank", name="tinybank")
            tiny_n = [0]

            def tiny():
                i = tiny_n[0]
                tiny_n[0] += 1
                assert i < 256
                return tinybank[:, i : i + 1]

            def load(ap, shape, tag, dt_=F32):
                t = sb.tile(list(shape), dt_, tag=tag, name=tag)
                nc.sync.dma_start(t[:], ap)
                return t

            def mset(ap_, val=0.0):
                if ap_.dtype == F32R:
                    ap_ = ap_.bitcast(F32)
                nc.vector.memset(ap_, val)

            # ---------- load constants & weights ----------
            ids_sb = load(idsr[:], (1, B * P), "idsr", F32R)
            midi_sb = load(midir[:], (1, T), "midir", F32R)
            cum_sb = load(cum2[:], (P, 2), "cum2")
            phemb_sb = load(phemb[:], (NUM_PH, E), "phemb", F32R)
            w1ph_sb = [load(w1ph[i], (128, 3, 2, 128), f"w1ph{i}", F32R) for i in range(2)]
            w2ph_sb = [load(w2ph[i], (128, 3, 2, 128), f"w2ph{i}", F32R) for i in range(2)]
            f0w1_sb = load(f0w1[:], (3, 128), "f0w1", F32R)
            f0w2_sb = load(f0w2[:], (128, 3, 2, 128), "f0w2", F32R)
            ptemb_sb = load(ptemb[:], (128, H), "ptemb", F32R)
            pcw_sb = [load(pcw[i], (128, 3, 2, 128), f"pcw{i}", F32R) for i in range(4)]
            cbw1_sb = [load(cbw1[i], (128, 3, 2, 128), f"pcw{i}", F32R) for i in range(4)]
            cbw2_sb = [load(cbw2[i], (128, 3, 2, 128), f"w1ph{i}", F32R) for i in range(2)]
            fpw_sb = [load(fpw[i], (128, 4, 128), f"w2ph{i}", F32R) for i in range(2)]
            fpb_sb = load(fpb[:], (128, 4), "fpb")
            aff_sb = load(bnaff[:], (128, 26), "bnaff")
            id_sb = load(ident[:], (128, 128), "ident")
            on_sb = load(ones1[:], (1, 128), "ones1", F32R)
            io_sb = load(iotac[:], (128, 1), "iotac")
            tr_sb = load(trow[:], (1, T), "trow", F32R)

            eps_sb = sb.tile([128, 1], F32, tag="eps", name="eps")
            nc.vector.memset(eps_sb[:], EPS)

            aff_idx = {}
            _i = 0
            for lname, nct in [("ph1", 2), ("ph2", 2), ("f01", 1), ("f02", 2),
                               ("pc", 2), ("cb1", 2), ("cb2", 2)]:
                aff_idx[lname] = _i
                _i += 2 * nct

            def aff_cols(lname, nct, ct):
                base = aff_idx[lname]
                g = aff_sb[:, base + ct : base + ct + 1]
                be = aff_sb[:, base + nct + ct : base + nct + ct + 1]
                return g, be

            def bn_coeffs(gsum, gsq, n, g_col, b_col):
                mean = tiny()
                nc.vector.tensor_scalar_mul(mean[:], gsum, 1.0 / n)
                ex2 = tiny()
                nc.vector.tensor_scalar_mul(ex2[:], gsq, 1.0 / n)
                m2 = tiny()
                nc.scalar.activation(m2[:], mean[:], ACTF.Square)
                var = tiny()
                nc.vector.tensor_sub(var[:], ex2[:], m2[:])
                std = tiny()
                nc.scalar.activation(std[:], var[:], ACTF.Sqrt, bias=eps_sb[:])
                inv = tiny()
                nc.vector.reciprocal(inv[:], std[:])
                scale = tiny()
                nc.vector.tensor_mul(scale[:], inv[:], g_col)
                mb = tiny()
                nc.vector.tensor_mul(mb[:], mean[:], scale[:])
                bias = tiny()
                nc.vector.tensor_sub(bias[:], b_col, mb[:])
                return scale, bias

            def leaky_inplace(ap_):
                nc.vector.scalar_tensor_tensor(ap_, ap_, 0.1, ap_, ALU.mult, ALU.max)

            st_dumps = {}

            def ar_stats(tag, st):
                """AllReduce the [128, 4] sum/sumsq tile; returns global tile."""
                arin = drp.tile([128, 4], F32, tag=f"arin_{tag}", name=f"arin_{tag}")
                arout = drp.tile([128, 4], F32, tag=f"arout_{tag}", name=f"arout_{tag}")
                nc.gpsimd.dma_start(arin[:], st[:])
                nc.gpsimd.collective_compute(
                    "AllReduce", ALU.add,
                    replica_groups=[list(range(N_CORES))],
                    ins=[arin.opt()], outs=[arout.opt()],
                )
                gst = sb.tile([128, 4], F32, tag=f"gst_{tag}", name=f"gst_{tag}")
                nc.gpsimd.dma_start(gst[:], arout[:])
                st_dumps[tag] = gst
                return gst

            # Warm up the collective path (ncfw/descriptor staging) so the
            # real AllReduces are low-latency.
            warm = sb.tile([128, 1], F32, tag="warm", name="warm")
            nc.vector.memset(warm[:], 0.0)
            win = drp.tile([128, 1], F32, tag="warm_in", name="warm_in")
            wout = drp.tile([128, 1], F32, tag="warm_out", name="warm_out")
            nc.gpsimd.dma_start(win[:], warm[:])
            nc.gpsimd.collective_compute(
                "AllReduce", ALU.add, replica_groups=[list(range(N_CORES))],
                ins=[win.opt()], outs=[wout.opt()],
            )

            # =========================================================
            # f0 conv1 (full batch, replicated; local BN stats;
            # only own-b output is materialized)
            # =========================================================
            f0sh = sb.tile([3, B, T], F32R, tag="f0sh", name="f0sh")
            mset(f0sh[:, :, 0:1])
            mset(f0sh[:, :, T - 1 : T])
            f0r3 = f0r[:].rearrange("(o b) t -> o b t", o=1)
            nc.sync.dma_start(f0sh[0:1, :, 1:T], f0r3[:, :, 0 : T - 1])
            nc.sync.dma_start(f0sh[1:2, :, :], f0r3[:, :, :])
            nc.sync.dma_start(f0sh[2:3, :, 0 : T - 1], f0r3[:, :, 1:T])
            nc.scalar.activation(f0sh[:], f0sh[:], ACTF.Relu, scale=1.0 / 500.0)

            g1p = sb.tile([128, T + 2], F32R, tag="g1p", name="g1p")  # own b only
            mset(g1p[:, 0:1])
            mset(g1p[:, T + 1 : T + 2])
            c1sum = sb.tile([128, 16], F32, tag="c1sum", name="c1sum")
            c1sq = sb.tile([128, 16], F32, tag="c1sq", name="c1sq")
            for bb in range(B):
                for hh in range(2):
                    i = bb * 2 + hh
                    ps = pss.tile([128, 512], F32, tag="ps", name="ps")
                    nc.tensor.matmul(ps[:], f0w1_sb[:],
                                     f0sh[0:3, bb, hh * 512 : hh * 512 + 512])
                    if bb == 0:
                        nc.vector.tensor_scalar(
                            g1p[:, 1 + hh * 512 : 513 + hh * 512], ps[:],
                            0.0, None, ALU.add, ALU.add,
                            accum_out=c1sum[:, i : i + 1])
                    else:
                        nc.vector.tensor_reduce(c1sum[:, i : i + 1], ps[:],
                                                axis=mybir.AxisListType.X, op=ALU.add)
                    nc.scalar.activation(ps[:], ps[:], ACTF.Square,
                                         accum_out=c1sq[:, i : i + 1])
            s1 = tiny()
            nc.vector.reduce_sum(s1[:], c1sum[:], axis=mybir.AxisListType.X)
            q1 = tiny()
            nc.vector.reduce_sum(q1[:], c1sq[:], axis=mybir.AxisListType.X)
            g_c, b_c = aff_cols("f01", 1, 0)
            sc, bi = bn_coeffs(s1[:], q1[:], B * T, g_c, b_c)
            gint = g1p[:, 1 : T + 1]
            nc.scalar.activation(gint, gint, ACTF.Identity, bias=bi[:], scale=sc[:])
            leaky_inplace(gint)

            # =========================================================
            # f0 conv2 (own b only, stats AllReduced)
            # =========================================================
            g2raw = [sb.tile([128, T], F32, tag=f"g2raw{ct}", name=f"g2raw{ct}")
                     for ct in range(2)]
            st_f02 = sb.tile([128, 4], F32, tag="st_f02", name="st_f02")
            for ct in range(2):
                for hh in range(2):
                    ps = pss.tile([128, 512], F32, tag="ps", name="ps")
                    for k in range(3):
                        nc.tensor.matmul(
                            ps[:], f0w2_sb[:, k, ct, :],
                            g1p[:, hh * 512 + k : hh * 512 + k + 512],
                            start=(k == 0), stop=(k == 2))
                    nc.vector.tensor_scalar(
                        g2raw[ct][:, hh * 512 : hh * 512 + 512], ps[:],
                        0.0, None, ALU.add, ALU.add,
                        accum_out=c1sum[:, 8 + ct * 2 + hh : 9 + ct * 2 + hh])
                    nc.scalar.activation(ps[:], ps[:], ACTF.Square,
                                         accum_out=c1sq[:, 8 + ct * 2 + hh : 9 + ct * 2 + hh])
            for ct in range(2):
                nc.vector.reduce_sum(st_f02[:, 2 * ct : 2 * ct + 1],
                                     c1sum[:, 8 + 2 * ct : 10 + 2 * ct],
                                     axis=mybir.AxisListType.X)
                nc.vector.reduce_sum(st_f02[:, 2 * ct + 1 : 2 * ct + 2],
                                     c1sq[:, 8 + 2 * ct : 10 + 2 * ct],
                                     axis=mybir.AxisListType.X)
            gst_f02 = ar_stats("f02", st_f02)

            # =========================================================
            # pitch embedding for own midi row (fills the f02-AR window)
            # =========================================================
            pcin = [sb.tile([128, T + 2], F32R, tag=f"pcin{i}", name=f"pcin{i}")
                    for i in range(4)]
            for i in range(4):
                mset(pcin[i][:, 0:1])
                mset(pcin[i][:, T + 1 : T + 2])
            opt_ = sb.tile([128, T], F32R, tag="opt", name="opt")
            for hh in range(2):
                ps = pss.tile([128, 512], F32, tag="ps", name="ps")
                nc.tensor.matmul(ps[:], on_sb[:], midi_sb[:, hh * 512 : hh * 512 + 512])
                nc.vector.tensor_scalar(opt_[:, hh * 512 : hh * 512 + 512], ps[:],
                                        io_sb[:], None, ALU.is_equal)
            for ht in range(2):
                for tc_ in range(2):
                    ps = pss.tile([128, 512], F32, tag="ps", name="ps")
                    nc.tensor.matmul(ps[:], ptemb_sb[:, ht * 128 : ht * 128 + 128],
                                     opt_[:, tc_ * 512 : tc_ * 512 + 512])
                    nc.vector.tensor_copy(
                        pcin[ht][:, 1 + tc_ * 512 : 513 + tc_ * 512], ps[:])

            # =========================================================
            # phoneme encoder conv1 (full batch; local stats)
            # =========================================================
            psI = pss.tile([128, 512], F32, tag="ps", name="ps")
            nc.tensor.matmul(psI[0:NUM_PH, :], on_sb[:, 0:NUM_PH], ids_sb[:])
            oph = sb.tile([NUM_PH, B * P], F32R, tag="oph", name="oph")
            nc.vector.tensor_scalar(oph[:], psI[0:NUM_PH, :], io_sb[0:NUM_PH, :],
                                    None, ALU.is_equal)
            x0p = [sb.tile([128, B, P + 2], F32R, tag=f"x0p{i}", name=f"x0p{i}")
                   for i in range(2)]
            x1p = [sb.tile([128, B, P + 2], F32R, tag=f"x1p{i}", name=f"x1p{i}")
                   for i in range(2)]
            for i in range(2):
                mset(x0p[i][:, :, 0:1])
                mset(x0p[i][:, :, P + 1 : P + 2])
                mset(x1p[i][:, :, 0:1])
                mset(x1p[i][:, :, P + 1 : P + 2])
            for et in range(2):
                ps = pss.tile([128, 512], F32, tag="ps", name="ps")
                nc.tensor.matmul(ps[:], phemb_sb[:, et * 128 : et * 128 + 128], oph[:])
                nc.vector.tensor_copy(
                    x0p[et][:, :, 1 : P + 1],
                    ps[:].rearrange("p (b l) -> p b l", b=B))
            phsum = sb.tile([128, 4], F32, tag="phsum", name="phsum")
            phsq = sb.tile([128, 4], F32, tag="phsq", name="phsq")
            for ct in range(2):
                ps = pss.tile([128, 512], F32, tag="ps", name="ps")
                psv = ps[:].rearrange("p (b l) -> p b l", b=B)
                n_mm = 0
                for et in range(2):
                    for k in range(3):
                        nc.tensor.matmul(
                            psv, w1ph_sb[et][:, k, ct, :],
                            x0p[et][:, :, k : k + P],
                            start=(n_mm == 0), stop=(n_mm == 5))
                        n_mm += 1
                nc.vector.tensor_scalar(
                    x1p[ct][:, :, 1 : P + 1], psv, 0.0, None, ALU.add, ALU.add,
                    accum_out=phsum[:, ct : ct + 1])
                nc.scalar.activation(ps[:], ps[:], ACTF.Square,
                                     accum_out=phsq[:, ct : ct + 1])
            for ct in range(2):
                g_c, b_c = aff_cols("ph1", 2, ct)
                sc, bi = bn_coeffs(phsum[:, ct : ct + 1], phsq[:, ct : ct + 1],
                                   B * P, g_c, b_c)
                dst = x1p[ct][:, :, 1 : P + 1]
                nc.scalar.activation(dst, dst, ACTF.Relu, bias=bi[:], scale=sc[:])

            # ---- f0 conv2 BN apply (after its AR) -> pcin[2:4]
            for ct in range(2):
                g_c, b_c = aff_cols("f02", 2, ct)
                sc, bi = bn_coeffs(gst_f02[:, 2 * ct : 2 * ct + 1],
                                   gst_f02[:, 2 * ct + 1 : 2 * ct + 2],
                                   N_CORES * T, g_c, b_c)
                dst = pcin[2 + ct][:, 1 : T + 1]
                nc.scalar.activation(dst, g2raw[ct][:], ACTF.Identity,
                                     bias=bi[:], scale=sc[:])
                leaky_inplace(dst)

            # =========================================================
            # pc conv (own b; stats AllReduced)
            # =========================================================
            def conv_mm(w_sb, in_tiles, n_ci, st):
                pbt = []
                for ct in range(2):
                    pb = psb.tile([128, T], F32, tag="pb", name="pb")
                    pbt.append(pb)
                    for tc_ in range(2):
                        n_mm = 0
                        for ci in range(n_ci):
                            for k in range(3):
                                nc.tensor.matmul(
                                    pb[:, tc_ * 512 : tc_ * 512 + 512],
                                    w_sb[ci][:, k, ct, :],
                                    in_tiles[ci][:, tc_ * 512 + k : tc_ * 512 + k + 512],
                                    start=(n_mm == 0), stop=(n_mm == 3 * n_ci - 1))
                                n_mm += 1
                    nc.vector.tensor_reduce(st[:, 2 * ct : 2 * ct + 1], pb[:],
                                            axis=mybir.AxisListType.X, op=ALU.add)
                    scr = stg.tile([128, T], F32, tag="scr1024", name="scr1024", bufs=2)
                    nc.scalar.activation(scr[:], pb[:], ACTF.Square,
                                         accum_out=st[:, 2 * ct + 1 : 2 * ct + 2])
                return pbt

            def conv_apply(pbt, gst, lname, out_pad_tiles, interior):
                for ct in range(2):
                    g_c, b_c = aff_cols(lname, 2, ct)
                    sc, bi = bn_coeffs(gst[:, 2 * ct : 2 * ct + 1],
                                       gst[:, 2 * ct + 1 : 2 * ct + 2],
                                       N_CORES * T, g_c, b_c)
                    dst = out_pad_tiles[ct][:, interior[0] : interior[1]]
                    nc.scalar.activation(dst, pbt[ct][:], ACTF.Identity,
                                         bias=bi[:], scale=sc[:])
                    leaky_inplace(dst)

            st_pc = sb.tile([128, 4], F32, tag="st_pc", name="st_pc")
            pb_pc = conv_mm(pcw_sb, pcin, 4, st_pc)
            gst_pc = ar_stats("pc", st_pc)

            # =========================================================
            # phoneme conv2 + length regulator (fills the pc-AR window)
            # =========================================================
            x2sb = [sb.tile([128, B, P], F32, tag=f"x2sb{i}", name=f"x2sb{i}")
                    for i in range(2)]
            for ct in range(2):
                ps = pss.tile([128, 512], F32, tag="ps", name="ps")
                psv = ps[:].rearrange("p (b l) -> p b l", b=B)
                n_mm = 0
                for et in range(2):
                    for k in range(3):
                        nc.tensor.matmul(
                            psv, w2ph_sb[et][:, k, ct, :],
                            x1p[et][:, :, k : k + P],
                            start=(n_mm == 0), stop=(n_mm == 5))
                        n_mm += 1
                nc.vector.tensor_scalar(
                    x2sb[ct][:], psv, 0.0, None, ALU.add, ALU.add,
                    accum_out=phsum[:, 2 + ct : 3 + ct])
                nc.scalar.activation(ps[:], ps[:], ACTF.Square,
                                     accum_out=phsq[:, 2 + ct : 3 + ct])
            x2T = sb.tile([P, H], F32R, tag="x2T", name="x2T")
            for ct in range(2):
                g_c, b_c = aff_cols("ph2", 2, ct)
                sc, bi = bn_coeffs(phsum[:, 2 + ct : 3 + ct], phsq[:, 2 + ct : 3 + ct],
                                   B * P, g_c, b_c)
                # BN+relu only for own b (index 0) -- that's all we expand
                nc.scalar.activation(x2sb[ct][:, 0, :], x2sb[ct][:, 0, :], ACTF.Relu,
                                     bias=bi[:], scale=sc[:])
                ps = pss.tile([128, 512], F32, tag="ps", name="ps")
                nc.tensor.transpose(ps[0:P, 0:128], x2sb[ct][:, 0, :], id_sb[:])
                nc.vector.tensor_copy(x2T[:, ct * 128 : ct * 128 + 128], ps[0:P, 0:128])
            tbf = sb.tile([P, T], F32, tag="tbf", name="tbf")
            for hh in range(2):
                ps = pss.tile([128, 512], F32, tag="ps", name="ps")
                nc.tensor.matmul(ps[0:P, :], on_sb[:, 0:P],
                                 tr_sb[:, hh * 512 : hh * 512 + 512])
                nc.vector.tensor_copy(tbf[:, hh * 512 : hh * 512 + 512], ps[0:P, :])
            smat = sb.tile([P, T], F32R, tag="smat", name="smat")
            nc.vector.tensor_scalar(smat[:], tbf[:], cum_sb[:, 1:2], None, ALU.is_ge)
            nc.vector.scalar_tensor_tensor(smat[:], tbf[:], cum_sb[:, 0:1], smat[:],
                                           ALU.is_lt, ALU.mult)
            cb1in = [sb.tile([128, T + 2], F32R, tag=f"cb1in{i}", name=f"cb1in{i}")
                     for i in range(4)]
            for i in range(4):
                mset(cb1in[i][:, 0:1])
                mset(cb1in[i][:, T + 1 : T + 2])
            for ht in range(2):
                for tc_ in range(2):
                    ps = pss.tile([128, 512], F32, tag="ps", name="ps")
                    nc.tensor.matmul(ps[:], x2T[:, ht * 128 : ht * 128 + 128],
                                     smat[:, tc_ * 512 : tc_ * 512 + 512])
                    nc.vector.tensor_copy(
                        cb1in[ht][:, 1 + tc_ * 512 : 513 + tc_ * 512], ps[:])

            # =========================================================
            # cb1, cb2 convs (own b; stats AllReduced), fp 1x1
            # =========================================================
            cb2in = [sb.tile([128, T + 2], F32R, tag=f"x0p{i}", name=f"cb2in{i}")
                     for i in range(2)]
            for i in range(2):
                mset(cb2in[i][:, 0:1])
                mset(cb2in[i][:, T + 1 : T + 2])
            fpin = [sb.tile([128, T], F32R, tag=f"x1p{i}", name=f"fpin{i}")
                    for i in range(2)]

            conv_apply(pb_pc, gst_pc, "pc", cb1in[2:], (1, T + 1))
            st_cb1 = sb.tile([128, 4], F32, tag="st_cb1", name="st_cb1")
            pb_c1 = conv_mm(cbw1_sb, cb1in, 4, st_cb1)
            gst_c1 = ar_stats("cb1", st_cb1)
            conv_apply(pb_c1, gst_c1, "cb1", cb2in, (1, T + 1))
            st_cb2 = sb.tile([128, 4], F32, tag="st_cb2", name="st_cb2")
            pb_c2 = conv_mm(cbw2_sb, cb2in, 2, st_cb2)
            gst_c2 = ar_stats("cb2", st_cb2)
            conv_apply(pb_c2, gst_c2, "cb2", fpin, (0, T))

            ud_tags = ["g2raw0", "g2raw1", "x2sb0", "x2sb1"]
            ud = [sb.tile([128, T], F32, tag=ud_tags[i], name=f"ud{i}")
                  for i in range(4)]
            for g in range(2):
                for ct2 in range(2):
                    ct = g * 2 + ct2
                    pb = psb.tile([128, T], F32, tag="pb", name="pb")
                    for tc_ in range(2):
                        for ci in range(2):
                            nc.tensor.matmul(
                                pb[:, tc_ * 512 : tc_ * 512 + 512],
                                fpw_sb[ci][:, ct, :],
                                fpin[ci][:, tc_ * 512 : tc_ * 512 + 512],
                                start=(ci == 0), stop=(ci == 1))
                    nc.scalar.activation(ud[ct][:], pb[:], ACTF.Identity,
                                         bias=fpb_sb[:, ct : ct + 1])
            vv_tags = ["opt", "oph"]
            vv = [sb.tile([128, T], F32, tag=vv_tags[i], name=f"vv{i}")
                  for i in range(2)]
            for ht in range(2):
                nc.vector.tensor_add(vv[ht][:], ud[ht][:], ud[2 + ht][:])

            if debug:
                nc.sync.dma_start(dbg["d_oph"][:], oph[:].bitcast(F32))
                nc.sync.dma_start(dbg["d_opt"][:], opt_[:].bitcast(F32))
                nc.sync.dma_start(dbg["d_smat"][:], smat[:].bitcast(F32))
                nc.sync.dma_start(dbg["d_g1p"][:], g1p[:, 1 : T + 1].bitcast(F32))
                for i in range(4):
                    nc.sync.dma_start(dbg["d_pcin"][i], pcin[i][:, 1 : T + 1].bitcast(F32))
                    nc.sync.dma_start(dbg["d_cb1in"][i], cb1in[i][:, 1 : T + 1].bitcast(F32))
                    nc.sync.dma_start(dbg["d_ud"][i], ud[i][:])
                for i in range(2):
                    nc.sync.dma_start(dbg["d_cb2in"][i], cb2in[i][:, 1 : T + 1].bitcast(F32))
                    nc.sync.dma_start(dbg["d_fpin"][i], fpin[i][:].bitcast(F32))
                nc.sync.dma_start(dbg["d_st"][0], st_dumps["pc"][:])
                nc.sync.dma_start(dbg["d_st"][1], st_dumps["cb1"][:])
                nc.sync.dma_start(dbg["d_st"][2], st_dumps["cb2"][:])

            # =========================================================
            # interp + write out: out[h, f, t] = u + W1[f] * d
            # =========================================================
            for ht in range(2):
                u, d, v = ud[ht], ud[2 + ht], vv[ht]
                for f in range(FREQ):
                    w1f = float(W1[f])
                    if w1f == 0.0:
                        src = u
                    elif w1f == 1.0:
                        src = v
                    else:
                        s = stg.tile([128, T], F32, tag="stage", name="stage")
                        nc.vector.scalar_tensor_tensor(s[:], d[:], w1f, u[:],
                                                       ALU.mult, ALU.add)
                        src = s
                    nc.sync.dma_start(out[ht * 128 : ht * 128 + 128, f, :], src[:])

    return dram


_COMPILED = {}


def _get_compiled(debug=False):
    if debug not in _COMPILED:
        nc = bacc.Bacc("TRN2", target_bir_lowering=False, debug=False,
                       num_devices=N_CORES)
        _build(nc, debug=debug)
        nc.compile()
        _COMPILED[debug] = nc
    return _COMPILED[debug]


def _host_prep(phoneme_ids, phoneme_durations, midi_pitch, f0, params):
    p = {k: np.asarray(v, dtype=np.float32) for k, v in params.items()}
    ids = np.asarray(phoneme_ids).astype(np.int64)
    dur = np.asarray(phoneme_durations).astype(np.int64)
    midi = np.asarray(midi_pitch).astype(np.int64)
    f0 = np.asarray(f0, dtype=np.float32)

    def conv_lhsT(w, n_ci, n_ct):
        wt = np.ascontiguousarray(np.transpose(w, (1, 2, 0)))  # [cin, k, cout]
        return wt.reshape(n_ci, 128, 3, n_ct, 128)

    shared = {
        "phemb": p["ph_emb"],
        "w1ph": conv_lhsT(p["ph_w1"], 2, 2),
        "w2ph": conv_lhsT(p["ph_w2"], 2, 2),
        "f0w1": np.ascontiguousarray(p["f0_w1"][:, 0, :].T),
        "f0w2": conv_lhsT(p["f0_w2"], 1, 2).reshape(128, 3, 2, 128),
        "ptemb": p["pt_emb"],
        "pcw": conv_lhsT(p["pc_w"], 4, 2),
        "cbw1": conv_lhsT(p["cb_w1"], 4, 2),
        "cbw2": conv_lhsT(p["cb_w2"], 2, 2),
        "ident": np.eye(128, dtype=np.float32),
        "ones1": np.ones((1, 128), dtype=np.float32),
        "iotac": np.arange(128, dtype=np.float32).reshape(128, 1),
        "trow": np.arange(T, dtype=np.float32).reshape(1, T),
    }
    fpT = p["fp_w"][:, :, 0].T  # [cin 256, cout 512]
    fpmod = np.concatenate([fpT[:, :H], fpT[:, H:] - fpT[:, :H]], axis=1)
    shared["fpw"] = np.ascontiguousarray(fpmod).reshape(2, 128, 4, 128)
    fb = np.concatenate([p["fp_b"][:H], p["fp_b"][H:] - p["fp_b"][:H]])
    shared["fpb"] = np.ascontiguousarray(fb.reshape(4, 128).T)

    rows = []
    for gk, bk, nct in [("ph_g1", "ph_be1", 2), ("ph_g2", "ph_be2", 2),
                        ("f0_g1", "f0_be1", 1), ("f0_g2", "f0_be2", 2),
                        ("pc_g", "pc_be", 2), ("cb_g1", "cb_be1", 2),
                        ("cb_g2", "cb_be2", 2)]:
        rows.append(p[gk].reshape(nct, 128))
        rows.append(p[bk].reshape(nct, 128))
    shared["bnaff"] = np.ascontiguousarray(np.concatenate(rows, 0).T)  # [128, 26]

    in_maps = []
    for c in range(N_CORES):
        rot = [(c + i) % B for i in range(B)]
        cum = np.maximum(dur[c], 0).cumsum().astype(np.float32)
        cumprev = np.concatenate([[0.0], cum[:-1]]).astype(np.float32)
        m = dict(shared)
        m["f0r"] = np.ascontiguousarray(f0[rot])
        m["idsr"] = np.ascontiguousarray(ids[rot].reshape(1, B * P)).astype(np.float32)
        m["midir"] = np.clip(midi[c], 0, 127).reshape(1, T).astype(np.float32)
        m["cum2"] = np.stack([cum, cumprev], axis=1)
        in_maps.append({k: np.ascontiguousarray(v, dtype=np.float32)
                        for k, v in m.items()})
    return in_maps


def kernel(phoneme_ids, phoneme_durations, midi_pitch, f0, mel_shape, params,
           _trace=False, _trace_kwargs=None):
    nc = _get_compiled()
    in_maps = _host_prep(phoneme_ids, phoneme_durations, midi_pitch, f0, params)
    res = bass_utils.run_bass_kernel_spmd(
        nc, in_maps, core_ids=list(range(N_CORES)), trace=_trace,
        **(_trace_kwargs or {}),
    )
    out = np.stack([res.results[c]["out"] for c in range(N_CORES)], axis=0)
    if _trace:
        kernel._last_result = res
    return out
